# revision 23
# baseline (speedup 1.0000x reference)
"""VQ codebook-lookup kernel for Trainium2 (8 NeuronCores, data-parallel).

Strategy: shard z along the batch dim (2 batches per core), replicate the
1024x64 codebook. Per 128-token tile:
  zsq = sum(z^2)    (PE transpose + fused square/sum custom DVE op)
  A   = zsq + esq   (ACT Identity with per-partition bias; esq broadcast once)
  s2  = 2*z.e       (PE fp32 matmul vs pre-doubled transposed codebook)
  nd  = s2 - A      (custom DVE op, row-max fused into the loss column)
  idx = first index where nd == max  (custom DVE op == reference argmin)
  z_q = codebook[idx]                (GPSIMD indirect DMA gather)
  out = zp + (z_q - zp)              (fused straight-through rounding)
The arithmetic order replicates the eager XLA-Neuron reference bit-exactly.
"""
import numpy as np
from contextlib import ExitStack

import concourse.bass as bass
import concourse.bacc as bacc
import concourse.dve_ops as dve_ops
import concourse.tile as tile
from concourse import mybir
from concourse.bass_utils import run_bass_kernel_spmd
from concourse.dve_spec import (
    Spec, Src0, Src1, C0, C1, AluOp, MaxNeg, Zero, Idx, select, eq, sq, lower,
    _has_src1,
)
from concourse.dve_uop import DveOpSpec
from concourse.masks import make_identity

F32 = mybir.dt.float32
U32 = mybir.dt.uint32
AX = mybir.AxisListType
ALU = mybir.AluOpType
ACTF = mybir.ActivationFunctionType

B, C, H, W = 16, 64, 64, 64
HW = H * W                      # 4096 tokens per batch
N_CORES = 8
NB = B // N_CORES               # batches per core = 2
TOK = 128                       # tokens per tile
TILES_PER_B = HW // TOK         # 32
N_E = 1024                      # codebook entries
BETA = 0.25
FLT_MAX = 3.4028234663852886e38


def _register(name, spec, subdim=False):
    for op in dve_ops.OPS:
        if op.name == name:
            return op
    op = dve_ops.DveOp(name, spec, subdim, uops_sha={})
    dve_ops.OPS.append(op)
    dve_ops.CUSTOM_DVE_SPECS[name] = spec
    row = dve_ops._CUSTOM_DVE_ROW_BASE + len(dve_ops.OPS) - 1
    dve_ops._SUB_OPCODE_FOR_NAME[name] = row
    for ver in ("v3", "v4"):
        try:
            s = DveOpSpec(name=name, opcode=row, uops=lower(spec, ver=ver),
                          rd1_en=_has_src1(spec)).sha(ver)
            op.uops_sha[ver] = s
        except Exception:
            pass
    return op


# out = in0 - in1 (exact IEEE); accum_out = max over free dim
VQ_SUBMAX = _register(
    "ANT_VQ_SUBMAX", Spec(body=Src0 - Src1, accum=AluOp.MAX, accum_init=MaxNeg))
# out = select(in0 == s0, idx, +FLT_MAX); accum_out(min) = first index of s0
VQ_IDXOF = _register(
    "ANT_VQ_IDXOF", Spec(body=select(eq(Src0, C0), Idx, Zero - MaxNeg),
                         accum=AluOp.MIN, accum_init=C1))
# out = in0^2; accum_out = sum over free dim (matches reference reduce bits)
VQ_SQSUM = _register(
    "ANT_VQ_SQSUM", Spec(body=sq(Src0), accum=AluOp.ADD, accum_init=Zero))
# out = (in0 - in1) + in1 (straight-through double rounding)
VQ_ST = _register("ANT_VQ_ST", Spec(body=(Src0 - Src1) + Src1))


def _build(store_on_act=True, load_on_act=False, loop_bufs=8, share_ps=False, psb_bufs=None, store_on_pool=False, cast_pool=False, st_pool=False, psa_bufs=2, setup_psa=False, setup_dma_act=False, st_tc=False, cast_group=1, split_psa=False):
    nc = bacc.Bacc("TRN2", target_bir_lowering=False, debug=False)
    zs = nc.dram_tensor("zs", [NB, C, HW], F32, kind="ExternalInput")
    cb = nc.dram_tensor("cb", [N_E, C], F32, kind="ExternalInput")
    zq = nc.dram_tensor("zq", [NB, C, HW], F32, kind="ExternalOutput")
    partial = nc.dram_tensor("partial", [1, 1], F32, kind="ExternalOutput")

    n_tiles = NB * TILES_PER_B  # 64
    store_eng = nc.gpsimd if store_on_pool else (nc.scalar if store_on_act else nc.sync)
    load_eng_name = "scalar" if load_on_act else "sync"

    with tile.TileContext(nc) as tc, ExitStack() as ctx:
        setup = ctx.enter_context(tc.tile_pool(name="setup", bufs=1))
        loop = ctx.enter_context(tc.tile_pool(name="loop", bufs=loop_bufs))
        psA = ctx.enter_context(tc.tile_pool(name="psA", bufs=psa_bufs, space="PSUM"))
        psG = ctx.enter_context(tc.tile_pool(name="psG", bufs=1, space="PSUM")) if split_psa else psA
        _psb = psb_bufs if psb_bufs is not None else (3 if share_ps else 2)
        psB = ctx.enter_context(tc.tile_pool(name="psB", bufs=_psb, space="PSUM"))

        ident = setup.tile([128, 128], F32)
        make_identity(nc, ident[:])
        ones_row = setup.tile([1, 128], F32)
        nc.vector.memset(ones_row[:], 1.0)
        ones_col = setup.tile([128, 1], F32)
        nc.vector.memset(ones_col[:], 1.0)
        lcols = setup.tile([128, n_tiles], F32)
        scr = setup.tile([128, N_E], F32)      # write-only custom-op outputs
        scr64 = setup.tile([128, C], F32)

        # --- codebook setup: cbT2 = 2*cb.T [64,1024], esq_bc [128,1024] ---
        cbt_sb = setup.tile([C, N_E], F32)
        esq_col = setup.tile([128, 8], F32)
        for i in range(8):
            cbc = loop.tile([128, C], F32, tag="cbc")
            nc.sync.dma_start(cbc[:], cb[i * 128:(i + 1) * 128, :])
            tp = psB.tile([C, 128], F32, tag="s2")
            nc.tensor.transpose(tp[:], cbc[:], ident[:])
            nc.scalar.copy(cbt_sb[:, i * 128:(i + 1) * 128], tp[:])
            nc.vector._custom_dve(VQ_SQSUM, out=scr64[:], in0=cbc[:],
                                  accum_out=esq_col[:, i:i + 1])

        cbt_one = setup.tile([C, N_E], F32)
        nc.vector.tensor_copy(cbt_one[:], cbt_sb[:])
        cbt2 = setup.tile([C, N_E], F32)
        nc.scalar.mul(cbt2[:], cbt_one[:], 2.0)

        esq_row_sb = setup.tile([1, N_E], F32)
        for i in range(8):
            tpr = psB.tile([1, 128], F32, tag="s2")
            nc.tensor.transpose(tpr[:], esq_col[:, i:i + 1], ident[:])
            nc.scalar.copy(esq_row_sb[:, i * 128:(i + 1) * 128], tpr[:])
        esq_row_one = setup.tile([1, N_E], F32)
        nc.vector.tensor_copy(esq_row_one[:], esq_row_sb[:])

        esq_bc = setup.tile([128, N_E], F32)
        for h in range(2):
            bc_ps = psB.tile([128, 512], F32, tag="s2")
            nc.tensor.matmul(bc_ps[:], lhsT=ones_row[:],
                             rhs=esq_row_one[:, h * 512:(h + 1) * 512])
            nc.scalar.copy(esq_bc[:, h * 512:(h + 1) * 512], bc_ps[:])

        # --- main loop over 64 token tiles ---
        _groups = [None]
        for t in range(n_tiles):
            b, ti = divmod(t, TILES_PER_B)
            sl = slice(ti * TOK, (ti + 1) * TOK)

            zt = loop.tile([C, TOK], F32, tag="zt")
            getattr(nc, load_eng_name).dma_start(zt[:], zs[b, :, sl])

            # zsq (bit-exact vs reference jnp.sum(zf*zf, axis=1))
            ztT_ps = psA.tile([TOK, C], F32, tag="ztT")
            nc.tensor.transpose(ztT_ps[:], zt[:], ident[:C, :C])
            zsq = loop.tile([TOK, 1], F32, tag="zsq")
            nc.vector._custom_dve(VQ_SQSUM, out=scr64[:], in0=ztT_ps[:],
                                  accum_out=zsq[:])

            # A = fl(esq + zsq) on ACT (exact IEEE add via Identity bias)
            a_sb = loop.tile([TOK, N_E], F32, tag="a")
            nc.scalar.activation(a_sb[:], esq_bc[:], ACTF.Identity,
                                 bias=zsq[:, 0:1], scale=1.0)

            # s2 = 2*z.e on PE (fp32, bit-exact vs reference einsum*2)
            s2_ps = psB.tile([TOK, N_E], F32, tag="s2")
            for h in range(2):
                nc.tensor.matmul(s2_ps[:, h * 512:(h + 1) * 512], lhsT=zt[:],
                                 rhs=cbt2[:, h * 512:(h + 1) * 512])

            # nd = s2 - A (= -d_ref); row max -> loss column
            nd = loop.tile([TOK, N_E], F32, tag="nd")
            nc.vector._custom_dve(VQ_SUBMAX, out=nd[:], in0=s2_ps[:],
                                  in1=a_sb[:], accum_out=lcols[:, t:t + 1])

            # first-occurrence argmax == reference argmin
            g0 = t % cast_group
            if g0 == 0:
                fidxg = loop.tile([TOK, cast_group], F32, tag="fidx")
                uidxg = loop.tile([TOK, cast_group], U32, tag="uidx")
                _groups[0] = (fidxg, uidxg)
            fidxg, uidxg = _groups[0]
            nc.vector._custom_dve(VQ_IDXOF, out=scr[:], in0=nd[:],
                                  s0=lcols[:, t:t + 1], s1=FLT_MAX,
                                  accum_out=fidxg[:, g0:g0 + 1])
            if g0 == cast_group - 1:
                nc.vector.tensor_copy(uidxg[:], fidxg[:])
            uidx = uidxg

            # gather codebook rows
            gq = loop.tile([TOK, C], F32, tag="gq")
            nc.gpsimd.indirect_dma_start(
                out=gq[:], out_offset=None, in_=cb[:, :],
                in_offset=bass.IndirectOffsetOnAxis(ap=uidx[:, g0:g0 + 1], axis=0))

            # straight-through rounding + transpose back to [C, TOK]
            if st_tc:
                st_tc_sb = loop.tile([TOK, C], F32, tag="st_tc")
                nc.vector._custom_dve(VQ_ST, out=st_tc_sb[:], in0=gq[:],
                                      in1=ztT_ps[:])
                stT_ps = psA.tile([C, TOK], F32, tag="gqT")
                nc.tensor.transpose(stT_ps[:], st_tc_sb[:], ident[:])
                store_eng.dma_start(zq[b, :, sl], stT_ps[:])
                continue
            if split_psa:
                gqT_ps = psG.tile([C, TOK], F32, tag="gqT")
            else:
                gqT_ps = psA.tile([C, TOK], F32, tag="ztT" if share_ps else "gqT")
            nc.tensor.transpose(gqT_ps[:], gq[:], ident[:])
            st = loop.tile([C, TOK], F32, tag="st")
            if st_pool:
                gqT_sb = loop.tile([C, TOK], F32, tag="gqT_sb")
                nc.scalar.copy(gqT_sb[:], gqT_ps[:])
                diff = loop.tile([C, TOK], F32, tag="diff")
                nc.gpsimd.tensor_tensor(diff[:], gqT_sb[:], zt[:], op=ALU.subtract)
                nc.gpsimd.tensor_tensor(st[:], diff[:], zt[:], op=ALU.add)
            else:
                nc.vector._custom_dve(VQ_ST, out=st[:], in0=gqT_ps[:], in1=zt[:])

            store_eng.dma_start(zq[b, :, sl], st[:])

        # --- loss partial: sum of per-token max(nd) = -sum d_min ---
        lsum = setup.tile([128, 1], F32)
        nc.vector.reduce_sum(lsum[:], lcols[:], axis=AX.X)
        tot_ps = psA.tile([1, 1], F32, tag="ztT")
        nc.tensor.matmul(tot_ps[:], lhsT=lsum[:], rhs=ones_col[:])
        tot_sb = setup.tile([1, 1], F32)
        nc.scalar.copy(tot_sb[:], tot_ps[:])
        nc.sync.dma_start(partial[:, :], tot_sb[:])

    nc.finalize()
    return nc


_NC_CACHE = []


def kernel(z, codebook):
    z = np.ascontiguousarray(np.asarray(z), dtype=np.float32)
    cb = np.ascontiguousarray(np.asarray(codebook), dtype=np.float32)
    assert z.shape == (B, C, H, W) and cb.shape == (N_E, C)

    if not _NC_CACHE:
        _NC_CACHE.append(_build())
    nc = _NC_CACHE[0]

    in_maps = [{"zs": z[i * NB:(i + 1) * NB].reshape(NB, C, HW), "cb": cb}
               for i in range(N_CORES)]
    res = run_bass_kernel_spmd(nc, in_maps, core_ids=list(range(N_CORES)))

    zq = np.concatenate(
        [r["zq"].reshape(NB, C, H, W) for r in res.results], axis=0)
    total_neg = sum(float(r["partial"][0, 0]) for r in res.results)
    m = -total_neg / float(B * HW * C)   # mean((z_q - zp)^2)
    loss = np.float32(np.float32(BETA * m) + np.float32(m))
    return zq, loss


if __name__ == "__main__":
    rng = np.random.RandomState(0)
    z = rng.randn(B, C, H, W).astype(np.float32)
    cb = (rng.rand(N_E, C).astype(np.float32) - 0.5) * (2.0 / N_E)
    out, loss = kernel(z, cb)
    print("ok", out.shape, loss)


# revision 24
# speedup vs baseline: 1.0075x; 1.0075x over previous
"""VQ codebook-lookup kernel for Trainium2 (8 NeuronCores, data-parallel).

Strategy: shard z along the batch dim (2 batches per core), replicate the
1024x64 codebook. Per 128-token tile:
  zsq = sum(z^2)    (PE transpose + fused square/sum custom DVE op)
  A   = zsq + esq   (ACT Identity with per-partition bias; esq broadcast once)
  s2  = 2*z.e       (PE fp32 matmul vs pre-doubled transposed codebook)
  nd  = s2 - A      (custom DVE op, row-max fused into the loss column)
  idx = first index where nd == max  (custom DVE op == reference argmin)
  z_q = codebook[idx]                (GPSIMD indirect DMA gather)
  out = zp + (z_q - zp)              (fused straight-through rounding)
The arithmetic order replicates the eager XLA-Neuron reference bit-exactly.
"""
import numpy as np
from contextlib import ExitStack

import concourse.bass as bass
import concourse.bacc as bacc
import concourse.dve_ops as dve_ops
import concourse.tile as tile
from concourse import mybir
from concourse.bass_utils import run_bass_kernel_spmd
from concourse.dve_spec import (
    Spec, Src0, Src1, C0, C1, AluOp, MaxNeg, Zero, Idx, select, eq, sq, lower,
    _has_src1,
)
from concourse.dve_uop import DveOpSpec
from concourse.masks import make_identity

F32 = mybir.dt.float32
U32 = mybir.dt.uint32
AX = mybir.AxisListType
ALU = mybir.AluOpType
ACTF = mybir.ActivationFunctionType

B, C, H, W = 16, 64, 64, 64
HW = H * W                      # 4096 tokens per batch
N_CORES = 8
NB = B // N_CORES               # batches per core = 2
TOK = 128                       # tokens per tile
TILES_PER_B = HW // TOK         # 32
N_E = 1024                      # codebook entries
BETA = 0.25
FLT_MAX = 3.4028234663852886e38


def _register(name, spec, subdim=False):
    for op in dve_ops.OPS:
        if op.name == name:
            return op
    op = dve_ops.DveOp(name, spec, subdim, uops_sha={})
    dve_ops.OPS.append(op)
    dve_ops.CUSTOM_DVE_SPECS[name] = spec
    row = dve_ops._CUSTOM_DVE_ROW_BASE + len(dve_ops.OPS) - 1
    dve_ops._SUB_OPCODE_FOR_NAME[name] = row
    for ver in ("v3", "v4"):
        try:
            s = DveOpSpec(name=name, opcode=row, uops=lower(spec, ver=ver),
                          rd1_en=_has_src1(spec)).sha(ver)
            op.uops_sha[ver] = s
        except Exception:
            pass
    return op


# out = in0 - in1 (exact IEEE); accum_out = max over free dim
VQ_SUBMAX = _register(
    "ANT_VQ_SUBMAX", Spec(body=Src0 - Src1, accum=AluOp.MAX, accum_init=MaxNeg))
# out = select(in0 == s0, idx, +FLT_MAX); accum_out(min) = first index of s0
VQ_IDXOF = _register(
    "ANT_VQ_IDXOF", Spec(body=select(eq(Src0, C0), Idx, Zero - MaxNeg),
                         accum=AluOp.MIN, accum_init=C1))
# out = in0^2; accum_out = sum over free dim (matches reference reduce bits)
VQ_SQSUM = _register(
    "ANT_VQ_SQSUM", Spec(body=sq(Src0), accum=AluOp.ADD, accum_init=Zero))
# out = (in0 - in1) + in1 (straight-through double rounding)
VQ_ST = _register("ANT_VQ_ST", Spec(body=(Src0 - Src1) + Src1))


def _build(store_on_act=True, load_on_act=False, loop_bufs=10, share_ps=False, psb_bufs=None, store_on_pool=False, cast_pool=False, st_pool=False, psa_bufs=2, setup_psa=False, setup_dma_act=False, st_tc=False, cast_group=1, split_psa=False):
    nc = bacc.Bacc("TRN2", target_bir_lowering=False, debug=False)
    zs = nc.dram_tensor("zs", [NB, C, HW], F32, kind="ExternalInput")
    cb = nc.dram_tensor("cb", [N_E, C], F32, kind="ExternalInput")
    zq = nc.dram_tensor("zq", [NB, C, HW], F32, kind="ExternalOutput")
    partial = nc.dram_tensor("partial", [1, 1], F32, kind="ExternalOutput")

    n_tiles = NB * TILES_PER_B  # 64
    store_eng = nc.gpsimd if store_on_pool else (nc.scalar if store_on_act else nc.sync)
    load_eng_name = "scalar" if load_on_act else "sync"

    with tile.TileContext(nc) as tc, ExitStack() as ctx:
        setup = ctx.enter_context(tc.tile_pool(name="setup", bufs=1))
        loop = ctx.enter_context(tc.tile_pool(name="loop", bufs=loop_bufs))
        psA = ctx.enter_context(tc.tile_pool(name="psA", bufs=psa_bufs, space="PSUM"))
        psG = ctx.enter_context(tc.tile_pool(name="psG", bufs=1, space="PSUM")) if split_psa else psA
        _psb = psb_bufs if psb_bufs is not None else (3 if share_ps else 2)
        psB = ctx.enter_context(tc.tile_pool(name="psB", bufs=_psb, space="PSUM"))

        ident = setup.tile([128, 128], F32)
        make_identity(nc, ident[:])
        ones_row = setup.tile([1, 128], F32)
        nc.vector.memset(ones_row[:], 1.0)
        ones_col = setup.tile([128, 1], F32)
        nc.vector.memset(ones_col[:], 1.0)
        lcols = setup.tile([128, n_tiles], F32)
        scr = setup.tile([128, N_E], F32)      # write-only custom-op outputs
        scr64 = setup.tile([128, C], F32)

        # --- codebook setup: cbT2 = 2*cb.T [64,1024], esq_bc [128,1024] ---
        cbt_sb = setup.tile([C, N_E], F32)
        esq_col = setup.tile([128, 8], F32)
        for i in range(8):
            cbc = loop.tile([128, C], F32, tag="cbc")
            nc.sync.dma_start(cbc[:], cb[i * 128:(i + 1) * 128, :])
            tp = psB.tile([C, 128], F32, tag="s2")
            nc.tensor.transpose(tp[:], cbc[:], ident[:])
            nc.scalar.copy(cbt_sb[:, i * 128:(i + 1) * 128], tp[:])
            nc.vector._custom_dve(VQ_SQSUM, out=scr64[:], in0=cbc[:],
                                  accum_out=esq_col[:, i:i + 1])

        cbt_one = setup.tile([C, N_E], F32)
        nc.vector.tensor_copy(cbt_one[:], cbt_sb[:])
        cbt2 = setup.tile([C, N_E], F32)
        nc.scalar.mul(cbt2[:], cbt_one[:], 2.0)

        esq_row_sb = setup.tile([1, N_E], F32)
        for i in range(8):
            tpr = psB.tile([1, 128], F32, tag="s2")
            nc.tensor.transpose(tpr[:], esq_col[:, i:i + 1], ident[:])
            nc.scalar.copy(esq_row_sb[:, i * 128:(i + 1) * 128], tpr[:])
        esq_row_one = setup.tile([1, N_E], F32)
        nc.vector.tensor_copy(esq_row_one[:], esq_row_sb[:])

        esq_bc = setup.tile([128, N_E], F32)
        for h in range(2):
            bc_ps = psB.tile([128, 512], F32, tag="s2")
            nc.tensor.matmul(bc_ps[:], lhsT=ones_row[:],
                             rhs=esq_row_one[:, h * 512:(h + 1) * 512])
            nc.scalar.copy(esq_bc[:, h * 512:(h + 1) * 512], bc_ps[:])

        # --- main loop over 64 token tiles ---
        _groups = [None]
        for t in range(n_tiles):
            b, ti = divmod(t, TILES_PER_B)
            sl = slice(ti * TOK, (ti + 1) * TOK)

            zt = loop.tile([C, TOK], F32, tag="zt")
            getattr(nc, load_eng_name).dma_start(zt[:], zs[b, :, sl])

            # zsq (bit-exact vs reference jnp.sum(zf*zf, axis=1))
            ztT_ps = psA.tile([TOK, C], F32, tag="ztT")
            nc.tensor.transpose(ztT_ps[:], zt[:], ident[:C, :C])
            zsq = loop.tile([TOK, 1], F32, tag="zsq")
            nc.vector._custom_dve(VQ_SQSUM, out=scr64[:], in0=ztT_ps[:],
                                  accum_out=zsq[:])

            # A = fl(esq + zsq) on ACT (exact IEEE add via Identity bias)
            a_sb = loop.tile([TOK, N_E], F32, tag="a")
            nc.scalar.activation(a_sb[:], esq_bc[:], ACTF.Identity,
                                 bias=zsq[:, 0:1], scale=1.0)

            # s2 = 2*z.e on PE (fp32, bit-exact vs reference einsum*2)
            s2_ps = psB.tile([TOK, N_E], F32, tag="s2")
            for h in range(2):
                nc.tensor.matmul(s2_ps[:, h * 512:(h + 1) * 512], lhsT=zt[:],
                                 rhs=cbt2[:, h * 512:(h + 1) * 512])

            # nd = s2 - A (= -d_ref); row max -> loss column
            nd = loop.tile([TOK, N_E], F32, tag="nd")
            nc.vector._custom_dve(VQ_SUBMAX, out=nd[:], in0=s2_ps[:],
                                  in1=a_sb[:], accum_out=lcols[:, t:t + 1])

            # first-occurrence argmax == reference argmin
            g0 = t % cast_group
            if g0 == 0:
                fidxg = loop.tile([TOK, cast_group], F32, tag="fidx")
                uidxg = loop.tile([TOK, cast_group], U32, tag="uidx")
                _groups[0] = (fidxg, uidxg)
            fidxg, uidxg = _groups[0]
            nc.vector._custom_dve(VQ_IDXOF, out=scr[:], in0=nd[:],
                                  s0=lcols[:, t:t + 1], s1=FLT_MAX,
                                  accum_out=fidxg[:, g0:g0 + 1])
            if g0 == cast_group - 1:
                nc.vector.tensor_copy(uidxg[:], fidxg[:])
            uidx = uidxg

            # gather codebook rows
            gq = loop.tile([TOK, C], F32, tag="gq")
            nc.gpsimd.indirect_dma_start(
                out=gq[:], out_offset=None, in_=cb[:, :],
                in_offset=bass.IndirectOffsetOnAxis(ap=uidx[:, g0:g0 + 1], axis=0))

            # straight-through rounding + transpose back to [C, TOK]
            if st_tc:
                st_tc_sb = loop.tile([TOK, C], F32, tag="st_tc")
                nc.vector._custom_dve(VQ_ST, out=st_tc_sb[:], in0=gq[:],
                                      in1=ztT_ps[:])
                stT_ps = psA.tile([C, TOK], F32, tag="gqT")
                nc.tensor.transpose(stT_ps[:], st_tc_sb[:], ident[:])
                store_eng.dma_start(zq[b, :, sl], stT_ps[:])
                continue
            if split_psa:
                gqT_ps = psG.tile([C, TOK], F32, tag="gqT")
            else:
                gqT_ps = psA.tile([C, TOK], F32, tag="ztT" if share_ps else "gqT")
            nc.tensor.transpose(gqT_ps[:], gq[:], ident[:])
            st = loop.tile([C, TOK], F32, tag="st")
            if st_pool:
                gqT_sb = loop.tile([C, TOK], F32, tag="gqT_sb")
                nc.scalar.copy(gqT_sb[:], gqT_ps[:])
                diff = loop.tile([C, TOK], F32, tag="diff")
                nc.gpsimd.tensor_tensor(diff[:], gqT_sb[:], zt[:], op=ALU.subtract)
                nc.gpsimd.tensor_tensor(st[:], diff[:], zt[:], op=ALU.add)
            else:
                nc.vector._custom_dve(VQ_ST, out=st[:], in0=gqT_ps[:], in1=zt[:])

            store_eng.dma_start(zq[b, :, sl], st[:])

        # --- loss partial: sum of per-token max(nd) = -sum d_min ---
        lsum = setup.tile([128, 1], F32)
        nc.vector.reduce_sum(lsum[:], lcols[:], axis=AX.X)
        tot_ps = psA.tile([1, 1], F32, tag="ztT")
        nc.tensor.matmul(tot_ps[:], lhsT=lsum[:], rhs=ones_col[:])
        tot_sb = setup.tile([1, 1], F32)
        nc.scalar.copy(tot_sb[:], tot_ps[:])
        nc.sync.dma_start(partial[:, :], tot_sb[:])

    nc.finalize()
    return nc


_NC_CACHE = []


def kernel(z, codebook):
    z = np.ascontiguousarray(np.asarray(z), dtype=np.float32)
    cb = np.ascontiguousarray(np.asarray(codebook), dtype=np.float32)
    assert z.shape == (B, C, H, W) and cb.shape == (N_E, C)

    if not _NC_CACHE:
        _NC_CACHE.append(_build())
    nc = _NC_CACHE[0]

    in_maps = [{"zs": z[i * NB:(i + 1) * NB].reshape(NB, C, HW), "cb": cb}
               for i in range(N_CORES)]
    res = run_bass_kernel_spmd(nc, in_maps, core_ids=list(range(N_CORES)))

    zq = np.concatenate(
        [r["zq"].reshape(NB, C, H, W) for r in res.results], axis=0)
    total_neg = sum(float(r["partial"][0, 0]) for r in res.results)
    m = -total_neg / float(B * HW * C)   # mean((z_q - zp)^2)
    loss = np.float32(np.float32(BETA * m) + np.float32(m))
    return zq, loss


if __name__ == "__main__":
    rng = np.random.RandomState(0)
    z = rng.randn(B, C, H, W).astype(np.float32)
    cb = (rng.rand(N_E, C).astype(np.float32) - 0.5) * (2.0 / N_E)
    out, loss = kernel(z, cb)
    print("ok", out.shape, loss)
